# revision 22
# baseline (speedup 1.0000x reference)
"""Trainium2 Bass kernel for BehaviorLemming, v4.

Two fused stencil steps, data-parallel over batch (B=16 / 8 cores).
Layout: H rows in partitions, (channel, W) in free dim, 5 groups of 4ch.

v4 vs v3:
- World uploaded as fp16 from the host (plus an fp32 density channel,
  W-padded): the on-device fp32->fp16 conversion pass disappears and
  input DMA traffic roughly halves.
- All products and mask-chain ops emitted as scalar_tensor_tensor /
  tensor_scalar (InstTensorScalarPtr): 4x DVE mode for all-fp16 SBUF
  operands, 2x for fp32 -- twice the throughput of tensor_tensor.
- Density comparisons read W-padded tiles ([np, 514]) so each compare
  is a single full-width op (no wrap-piece ops).
- b16 (mask roll) and dA2 (density roll) via SBUF->SBUF DMA partition
  shifts on the main sets instead of PE matmuls; exact step-1 density
  via pure DVE adds (no fp32 matmuls). PE runs only the fp16 chains.
- Step-1 stay term folded in on DVE (w1b += R) instead of the I@R
  matmul chain: balances PE vs DVE.
"""

import numpy as np

import concourse.bacc as bacc
import concourse.mybir as mybir
import concourse.tile as tile
from concourse.bass_utils import run_bass_kernel_spmd

B, C, H, W = 16, 20, 512, 512
WP = W + 2
N_CORES = 8
B_PER_CORE = B // N_CORES
ELEM_ID = 3.0
F32 = mybir.dt.float32
F16 = mybir.dt.float16
U16 = mybir.dt.uint16
NCH = 4
NGRP = C // NCH
MAIN_OUT = 124
M_B1 = 32              # partition offset of batch-1 block in the merged set
M_NP = 52

N_SETS = 9


def QMODE(idx, step, g):
    """Q product split: channels on DVE vs Pool (mult is the only fast
    legal Pool op). Returns n channels on DVE (0..4)."""
    if idx >= N_SETS - 2:
        return 4          # tail: Pool drains slower
    if g in (0, 2, 4):
        return 1
    return 2


def RMODE(idx, step, g):
    """Stay-term via PE identity-matmul chain ("pe") or DVE post-add on
    the evacuated tile ("dve"). Merged set: PE-bound drain, DVE idle."""
    if idx == N_SETS - 1:
        return "dve"
    if idx == N_SETS - 2 and step == 2:
        return "dve"
    return "pe"

# knob: engine for the exact-density multiplies ("pool" or "dve")
EXACT_MULT_ENG = "pool"

# knob: engine for the density compares ("pool" or "dve")
CMP_ENG = "dve"

# knob: engine for the mask AND-chain and exact-density adds
CHAIN_ENG = "dve"
EXACT_ADD_ENG = "pool"


def _load_rows(nc, dst_tile, src_ap, row_start, n_rows, p0=0, nch=NCH):
    """Load n_rows (mod H, split at wrap) of src [nch,H,W] into dst
    partitions [p0, p0+n_rows), free dim = (c, w)."""
    s = row_start % H
    remaining = n_rows
    while remaining > 0:
        n = min(remaining, H - s)
        src = src_ap[:, s : s + n, :].rearrange("c h w -> h c w")
        nc.sync.dma_start(out=dst_tile[p0 : p0 + n, :].rearrange(
            "h (c w) -> h c w", c=nch), in_=src)
        p0 += n
        s = (s + n) % H
        remaining -= n


def _load_rows_2d(nc, dst_tile, src_ap, row_start, n_rows, p0=0,
                  c0=0, c1=WP):
    """Load n_rows (mod H) of src [H, WP] cols [c0,c1) into dst
    partitions [p0, p0+n_rows)."""
    s = row_start % H
    remaining = n_rows
    while remaining > 0:
        n = min(remaining, H - s)
        nc.sync.dma_start(out=dst_tile[p0 : p0 + n, 0 : c1 - c0],
                          in_=src_ap[s : s + n, c0:c1])
        p0 += n
        s = (s + n) % H
        remaining -= n


class SetCtx:
    """Per-set emission state."""

    def __init__(self, sd):
        self.sd = sd
        self.wb = None        # [np,10240] f16 world
        self.d32 = None       # [np,514] f32 padded density
        self.dA1 = None       # [np,514] f32 padded density, rows -1
        self.dB1 = None       # [np,512] f32 density, rows +1
        self.w1b = None       # [np,10240] f16 step-1 world
        self.w1dp = None      # [np,514] f32 exact padded step-1 density
        self.m1 = None        # (mp, b16) step-1 masks
        self.m2 = None


def build_kernel():
    nc = bacc.Bacc("TRN2", target_bir_lowering=False, debug=False,
                   num_devices=N_CORES)
    wd16 = nc.dram_tensor("world16", [B_PER_CORE, C, H, W], F16,
                          kind="ExternalInput").ap()
    dpad = nc.dram_tensor("dpad32", [B_PER_CORE, H, WP], F32,
                          kind="ExternalInput").ap()
    su16_d = nc.dram_tensor("su16", [128, 128], F16, kind="ExternalInput").ap()
    sd16_d = nc.dram_tensor("sd16", [128, 128], F16, kind="ExternalInput").ap()
    i16_d = nc.dram_tensor("i16", [128, 128], F16, kind="ExternalInput").ap()
    sum16_d = nc.dram_tensor("sum16", [M_NP, M_NP], F16,
                             kind="ExternalInput").ap()
    sdm16_d = nc.dram_tensor("sdm16", [M_NP, M_NP], F16,
                             kind="ExternalInput").ap()
    sum32_d = nc.dram_tensor("sum32", [M_NP, M_NP], F32,
                             kind="ExternalInput").ap()
    wm16_d = nc.dram_tensor("wm16", [M_NP, C, W], F16,
                            kind="ExternalInput").ap()
    dm32_d = nc.dram_tensor("dm32", [M_NP, WP], F32,
                            kind="ExternalInput").ap()
    dAm32_d = nc.dram_tensor("dAm32", [M_NP, WP], F32,
                             kind="ExternalInput").ap()
    dBm32_d = nc.dram_tensor("dBm32", [M_NP, W], F32,
                             kind="ExternalInput").ap()
    od = nc.dram_tensor("out16", [B_PER_CORE, C, H, W], F16,
                        kind="ExternalOutput").ap()

    al = mybir.AluOpType

    with tile.TileContext(nc) as tc:
        with (
            tc.tile_pool(name="const", bufs=1) as cpool,
            tc.tile_pool(name="dfp", bufs=2) as dpool,
            tc.tile_pool(name="wbp", bufs=2) as wbpool,
            tc.tile_pool(name="w1p", bufs=2) as w1pool,
            tc.tile_pool(name="mkp", bufs=2) as mkpool,
            tc.tile_pool(name="pqr", bufs=3) as pqrpool,
            tc.tile_pool(name="ogp", bufs=2) as ogpool,
            tc.tile_pool(name="pmain", bufs=4, space="PSUM") as pmain,
        ):
            su16 = cpool.tile([128, 128], F16)
            sd16 = cpool.tile([128, 128], F16)
            i16 = cpool.tile([128, 128], F16)
            sum16 = cpool.tile([M_NP, M_NP], F16)
            sdm16 = cpool.tile([M_NP, M_NP], F16)
            sum32 = cpool.tile([M_NP, M_NP], F32)
            z16 = cpool.tile([128, W], F16)
            z32 = cpool.tile([128, WP], F32)
            nc.gpsimd.memset(z16[0:1, :], 0.0)
            nc.gpsimd.memset(z32[0:1, :], 0.0)

            def load_consts():
                # Act-queue HWDGE: runs in parallel with the set-0 world
                # loads on SP
                for t, d in ((su16, su16_d), (sd16, sd16_d), (i16, i16_d),
                             (sum16, sum16_d), (sdm16, sdm16_d),
                             (sum32, sum32_d)):
                    nc.scalar.dma_start(out=t[:], in_=d)

            def stt(eng, out, in0, in1, op0=al.mult, op1=al.mult,
                    scalar=1.0):
                eng.scalar_tensor_tensor(out=out, in0=in0, scalar=scalar,
                                         in1=in1, op0=op0, op1=op1)

            def masks_rolls(st, step):
                """Phase 1: d/dA sources. Step 2: dA2 = roll(w1d,+1,H)."""
                sd = st.sd
                np_ = sd["np"]
                if step == 1:
                    return {"d": st.d32, "dA": st.dA1}
                d = st.w1dp
                dAt = mkpool.tile([128, WP], F32, tag="dA2")
                if sd.get("merged"):
                    psx = pmain.tile([np_, 2 * W], F32, tag="ps")
                    nc.tensor.matmul(out=psx[:, 0:WP], lhsT=sum32[:],
                                     rhs=d[0:np_, :], start=True, stop=True)
                    nc.scalar.copy(dAt[0:np_, :], psx[:, 0:WP])
                else:
                    # partition shift via SBUF->SBUF DMA on the DVE queue
                    # (w1dp is DVE-produced: no head-of-line stall); row 0
                    # zeroed -- its masks are never stored but must stay
                    # finite (NaN would poison matmul accumulation).
                    nc.sync.dma_start(out=dAt[1:128, :], in_=d[0:127, :])
                    nc.sync.dma_start(out=dAt[0:1, :], in_=z32[0:1, :])
                return {"d": d, "dA": dAt}

            def masks_cmps(st, step, mc):
                """Phase 2: density compares on W-padded tiles."""
                np_ = st.sd["np"]
                lo = 0 if step == 1 else 2
                d = mc["d"][0:np_, 1 : 1 + W]
                df = mc["d"][0:np_, lo : lo + W]
                dA = mc["dA"][0:np_, 1 : 1 + W]
                dfA = mc["dA"][0:np_, lo : lo + W]
                c1 = mkpool.tile([np_, W], F16, tag="c1")
                mx = mkpool.tile([np_, W], F32, tag="mx")
                c23 = mkpool.tile([np_, W], F16, tag="c23")
                eng = nc.gpsimd if CMP_ENG == "pool" else nc.vector
                eng.tensor_tensor(out=c1[:], in0=df, in1=d, op=al.is_ge)
                # (dA < d) & (dfA < d)  ==  max(dA, dfA) < d
                eng.tensor_tensor(out=mx[:], in0=dA, in1=dfA, op=al.max)
                eng.tensor_tensor(out=c23[:], in0=mx[:], in1=d, op=al.is_lt)
                mc.update(c1=c1, c23=c23)

            def masks_chain(st, step, mc):
                """Phase 3: AND-tree -> a16; b16 = roll(a,-1,H)."""
                sd = st.sd
                np_ = sd["np"]
                e = (st.wb if step == 1 else st.w1b)[0:np_, 0:W]
                # e3 via tensor_scalar: 4x DVE mode (all-fp16 SBUF)
                e3 = mkpool.tile([np_, W], F16, tag="e3")
                nc.vector.tensor_scalar(out=e3[:], in0=e, scalar1=ELEM_ID,
                                        scalar2=None, op0=al.is_equal)
                ceng = nc.gpsimd if CHAIN_ENG == "pool" else nc.vector
                c123 = mkpool.tile([np_, W], F16, tag="c123")
                ceng.tensor_tensor(out=c123[:], in0=mc["c1"][:],
                                   in1=mc["c23"][:], op=al.logical_and)
                mp = mkpool.tile([np_, 2 * W], F16, tag="mp")
                a16 = mp[:, 0:W]
                ceng.tensor_tensor(out=a16, in0=c123[:], in1=e3[:],
                                   op=al.logical_and)
                b16 = mkpool.tile([np_, W], F16, tag="b16")
                if sd.get("merged"):
                    # block-structured roll via matmul (zeroes boundaries)
                    psx = pmain.tile([np_, 2 * W], F32, tag="ps")
                    nc.tensor.matmul(out=psx[:, W : W + W], lhsT=sdm16[:],
                                     rhs=a16, start=True, stop=True)
                    nc.scalar.copy(b16[:], psx[:, W : W + W])
                else:
                    # b16[p] = a16[p+1]; p=127 zeroed
                    nc.sync.dma_start(out=b16[0:127, :], in_=mp[1:128, 0:W])
                    nc.sync.dma_start(out=b16[127:128, :], in_=z16[0:1, :])
                mc.update(a16=a16, b16=b16, mp=mp)

            def masks_fin(st, step, mc):
                """Phase 4: r16 -> m016 (after the b16 shift has landed)."""
                np_ = st.sd["np"]
                a16, b16 = mc["a16"], mc["b16"]
                r16 = mkpool.tile([np_, W], F16, tag="r16")
                reng = nc.gpsimd if CHAIN_ENG == "pool" else nc.vector
                reng.tensor_tensor(out=r16[:], in0=a16, in1=b16[:],
                                   op=al.logical_or)
                m016 = mc["mp"][:, W : 2 * W]
                nc.vector.tensor_scalar(out=m016, in0=r16[:], scalar1=1.0,
                                        scalar2=None, op0=al.is_lt)
                if step == 1:
                    st.m1 = (mc["mp"], b16)
                else:
                    st.m2 = (mc["mp"], b16)

            def exact1(st):
                """Exact fp32 step-1 density (feeds step-2 masks), via
                pure DVE multiply/adds; result W-padded in w1dp."""
                np_ = st.sd["np"]
                d = st.d32[0:np_, 1 : 1 + W]
                dA = st.dA1[0:np_, 1 : 1 + W]
                dB = st.dB1[0:np_, 0:W]
                mp, b16 = st.m1
                a16 = mp[:, 0:W]
                m016 = mp[:, W : 2 * W]
                t1 = mkpool.tile([np_, W], F32, tag="t1")
                t2 = mkpool.tile([np_, W], F32, tag="t2")
                t3 = mkpool.tile([np_, W], F32, tag="t3")
                meng = nc.gpsimd if EXACT_MULT_ENG == "pool" else nc.vector
                meng.tensor_tensor(out=t1[:], in0=a16, in1=dA, op=al.mult)
                meng.tensor_tensor(out=t2[:], in0=b16[:], in1=dB,
                                   op=al.mult)
                meng.tensor_tensor(out=t3[:], in0=m016, in1=d, op=al.mult)
                w1dp = w1pool.tile([128, WP], F32, tag="w1dp")
                s12 = mkpool.tile([np_, W], F32, tag="s12")
                aeng = nc.gpsimd if EXACT_ADD_ENG == "pool" else nc.vector
                aeng.tensor_tensor(out=s12[:], in0=t1[:], in1=t2[:],
                                   op=al.add)
                aeng.tensor_tensor(out=w1dp[0:np_, 1 : 1 + W],
                                   in0=s12[:], in1=t3[:], op=al.add)
                # circular W pads: col0 <- col512 (w=511), col513 <- col1
                nc.scalar.copy(w1dp[0:np_, 0:1], w1dp[0:np_, W : W + 1])
                nc.scalar.copy(w1dp[0:np_, W + 1 : W + 2],
                               w1dp[0:np_, 1:2])
                st.w1dp = w1dp

            def emit_group(st, step, g):
                """One fp16 stencil group-step: products, matmuls, evac."""
                sd = st.sd
                np_ = sd["np"]
                mp, b16 = st.m1 if step == 1 else st.m2
                src = (st.wb if step == 1 else st.w1b)[
                    0:np_, g * NCH * W : (g + 1) * NCH * W]
                src_v = src.rearrange("p (c w) -> p c w", c=NCH)
                b_b = b16[:].unsqueeze(1).broadcast_to([np_, NCH, W])
                fd = NCH * W
                # Q first: the su-chain consumes it before P/R are needed
                Q = pqrpool.tile([np_, fd], F16, tag="Q")
                Qv = Q[:].rearrange("p (c w) -> p c w", c=NCH)
                h = NCH // 2
                ndve = QMODE(sd["idx"], step, g)
                if ndve < NCH:
                    nc.gpsimd.tensor_tensor(out=Qv[:, ndve:NCH],
                                            in0=b_b[:, ndve:NCH],
                                            in1=src_v[:, ndve:NCH],
                                            op=al.mult)
                if ndve > 0:
                    nc.vector.tensor_tensor(out=Qv[:, 0:ndve],
                                            in0=b_b[:, 0:ndve],
                                            in1=src_v[:, 0:ndve],
                                            op=al.mult)
                # P and R as ONE double-wide op over the packed [a16|m016]
                # mask pair broadcast over channels.
                PR = pqrpool.tile([np_, 2 * fd], F16, tag="PR")
                PRv = PR[:].rearrange("p (k c w) -> p k c w", k=2, c=NCH)
                mp_b = mp.rearrange("p (k w) -> p k w", k=2).unsqueeze(
                    2).broadcast_to([np_, 2, NCH, W])
                src_b = src_v.unsqueeze(1).broadcast_to([np_, 2, NCH, W])
                nc.vector.tensor_tensor(out=PRv, in0=mp_b, in1=src_b,
                                        op=al.mult)
                P = PR[:, 0:fd]
                R = PR[:, fd : 2 * fd]
                if step == 1:
                    og = None
                    dst = st.w1b[0:np_, g * fd : (g + 1) * fd]
                else:
                    og = ogpool.tile([np_, fd], F16, tag="og")
                    dst = og[0:np_, :]
                rmode = RMODE(sd["idx"], step, g)
                if rmode == "dve":
                    ev = ogpool.tile([np_, fd], F16, tag="ev")
                    evac_dst = ev[0:np_, :]
                else:
                    ev = None
                    evac_dst = dst
                hw = 2 * W
                for (c0, c1) in ((0, 1), (2, 3)):
                    ps = pmain.tile([np_, hw], F32, tag="ps")
                    for c in (c0, c1):
                        r = slice((c - c0) * W, (c - c0 + 1) * W)
                        nc.tensor.matmul(out=ps[:, r], lhsT=sd["su16"],
                                         rhs=Q[:, c * W : (c + 1) * W],
                                         start=True, stop=False)
                        last = rmode == "pe"
                        nc.tensor.matmul(out=ps[:, r], lhsT=sd["sd16"],
                                         rhs=P[:, c * W : (c + 1) * W],
                                         start=False, stop=not last)
                        if last:
                            nc.tensor.matmul(out=ps[:, r], lhsT=sd["i16"],
                                             rhs=R[:, c * W : (c + 1) * W],
                                             start=False, stop=True)
                    nc.scalar.copy(evac_dst[:, c0 * W : (c1 + 1) * W], ps[:])
                if rmode == "dve":
                    # disjoint merge: R nonzero only where the chains are 0
                    nc.vector.tensor_tensor(out=dst, in0=ev[0:np_, :],
                                            in1=R, op=al.add)
                if og is not None:
                    sd["store"](og, g)

            def prep_A(sd):
                """Next-set loads: world (fp16) + density tiles (fp32)."""
                st = SetCtx(sd)
                st.wb = wbpool.tile([128, C * W], F16, tag="wb")
                st.d32 = dpool.tile([128, WP], F32, tag="d32")
                st.dA1 = dpool.tile([128, WP], F32, tag="dA1")
                st.dB1 = dpool.tile([128, W], F32, tag="dB1")
                sd["load"](st)
                return st

            def make_main_set(bi, si):
                r_out = si * MAIN_OUT

                def load(st):
                    # mask inputs first: they gate the next set's pipeline
                    _load_rows_2d(nc, st.d32, dpad[bi], r_out - 2, 128)
                    _load_rows_2d(nc, st.dA1, dpad[bi], r_out - 3, 128)
                    _load_rows(nc, st.wb, wd16[bi], r_out - 2, 128, nch=C)
                    _load_rows_2d(nc, st.dB1, dpad[bi], r_out - 1, 128,
                                  c0=1, c1=1 + W)

                def store(og, g):
                    dst = od[bi, g * NCH : (g + 1) * NCH,
                             r_out : r_out + MAIN_OUT, :]
                    nc.scalar.dma_start(
                        out=dst.rearrange("c h w -> h c w"),
                        in_=og[2 : 2 + MAIN_OUT, :].rearrange(
                            "h (c w) -> h c w", c=NCH))

                return {"np": 128, "idx": 4 * bi + si, "su16": su16[:],
                        "sd16": sd16[:], "i16": i16[:], "load": load,
                        "store": store}

            def make_merged_set():
                r_out = 4 * MAIN_OUT
                n_out = H - r_out        # 16

                def load(st):
                    # host pre-stages the merged-set tiles (blocks at the
                    # right partition offsets, zero gaps): 4 clean DMAs,
                    # no Pool memsets (whose cost scales with free size)
                    nc.sync.dma_start(out=st.d32[0:M_NP, :], in_=dm32_d)
                    nc.sync.dma_start(out=st.dA1[0:M_NP, :], in_=dAm32_d)
                    nc.sync.dma_start(
                        out=st.wb[0:M_NP, :].rearrange(
                            "p (c w) -> p c w", c=C), in_=wm16_d)
                    nc.sync.dma_start(out=st.dB1[0:M_NP, 0:W], in_=dBm32_d)

                def store(og, g):
                    for bi, p0 in ((0, 2), (1, M_B1 + 2)):
                        dst = od[bi, g * NCH : (g + 1) * NCH,
                                 r_out : r_out + n_out, :]
                        nc.scalar.dma_start(
                            out=dst.rearrange("c h w -> h c w"),
                            in_=og[p0 : p0 + n_out, :].rearrange(
                                "h (c w) -> h c w", c=NCH))

                return {"np": M_NP, "idx": N_SETS - 1, "su16": sum16[:],
                        "sd16": sdm16[:], "i16": i16[0:M_NP, 0:M_NP],
                        "merged": True, "load": load, "store": store}

            sets = [make_main_set(bi, si)
                    for bi in range(B_PER_CORE) for si in range(4)]
            sets.append(make_merged_set())

            # deep software pipeline. Per-engine queues run in emission
            # order, so long-latency chains (partition-shift DMAs, evac-
            # dependent compares) are split into phases and interleaved
            # between bulk product groups that hide their latency.
            st = prep_A(sets[0])
            load_consts()
            mc1 = masks_rolls(st, 1)
            masks_cmps(st, 1, mc1)
            masks_chain(st, 1, mc1)
            masks_fin(st, 1, mc1)
            exact1(st)
            st.w1b = w1pool.tile([128, C * W], F16, tag="w1b")
            for i in range(len(sets)):
                # phase B: step 1 with step-2 mask phases interleaved
                nxt = sets[i + 1] if i + 1 < len(sets) else None
                emit_group(st, 1, 0)
                mc2 = masks_rolls(st, 2)
                masks_cmps(st, 2, mc2)
                emit_group(st, 1, 1)
                masks_chain(st, 2, mc2)
                emit_group(st, 1, 2)
                masks_fin(st, 2, mc2)
                stn = prep_A(nxt) if nxt else None
                emit_group(st, 1, 3)
                if stn:
                    mc1 = masks_rolls(stn, 1)
                    masks_cmps(stn, 1, mc1)
                emit_group(st, 1, 4)
                # phase C: step 2 with next-set mask chain interleaved
                emit_group(st, 2, 0)
                if stn:
                    masks_chain(stn, 1, mc1)
                emit_group(st, 2, 1)
                if stn:
                    masks_fin(stn, 1, mc1)
                emit_group(st, 2, 2)
                if stn:
                    exact1(stn)
                    stn.w1b = w1pool.tile([128, C * W], F16, tag="w1b")
                emit_group(st, 2, 3)
                emit_group(st, 2, 4)
                st = stn

    nc.compile()
    return nc


def _shift_mats():
    su = np.zeros((128, 128), np.float16)   # out[m] = in[m-1]
    sdn = np.zeros((128, 128), np.float16)  # out[m] = in[m+1]
    for m in range(128):
        if m >= 1:
            su[m - 1, m] = 1.0
        if m <= 126:
            sdn[m + 1, m] = 1.0
    sum_ = np.zeros((M_NP, M_NP), np.float32)
    sdm = np.zeros((M_NP, M_NP), np.float32)
    for base in (0, M_B1):
        for m in range(20):
            if m >= 1:
                sum_[base + m - 1, base + m] = 1.0
            if m <= 18:
                sdm[base + m + 1, base + m] = 1.0
    return su, sdn, sum_, sdm


_NC_CACHE = {}


def kernel(world, rand_movement=None, rand_interact=None, rand_element=None,
           **_ignored):
    world = np.ascontiguousarray(world, dtype=np.float32)
    assert world.shape == (B, C, H, W), world.shape
    if "nc" not in _NC_CACHE:
        _NC_CACHE["nc"] = build_kernel()
    nc = _NC_CACHE["nc"]
    su, sdn, sum_, sdm = _shift_mats()
    i16 = np.eye(128, dtype=np.float16)
    world16 = world.astype(np.float16)
    d = world[:, 1]                                # [B,H,W] fp32 density
    dpad = np.concatenate([d[:, :, W - 1 :], d, d[:, :, :1]], axis=2)
    dpad = np.ascontiguousarray(dpad, dtype=np.float32)
    # host-staged merged-set tiles (last 16 rows of each batch + halo)
    r_out = 4 * MAIN_OUT
    n_rows = H - r_out + 4                         # 20
    in_maps = []
    for core in range(N_CORES):
        sl = slice(core * B_PER_CORE, (core + 1) * B_PER_CORE)
        w16c = world16[sl]
        dpc = dpad[sl]
        wm = np.zeros((M_NP, C, W), np.float16)
        dm = np.zeros((M_NP, WP), np.float32)
        dAm = np.zeros((M_NP, WP), np.float32)
        dBm = np.zeros((M_NP, W), np.float32)
        for bi, p0 in ((0, 0), (1, M_B1)):
            r0 = np.arange(r_out - 2, r_out - 2 + n_rows) % H
            wm[p0 : p0 + n_rows] = w16c[bi].transpose(1, 0, 2)[r0]
            dm[p0 : p0 + n_rows] = dpc[bi][r0]
            dAm[p0 : p0 + n_rows] = dpc[bi][(r0 - 1) % H]
            dBm[p0 : p0 + n_rows] = dpc[bi][(r0 + 1) % H][:, 1 : 1 + W]
        in_maps.append({
            "world16": np.ascontiguousarray(w16c),
            "dpad32": dpc,
            "su16": su, "sd16": sdn, "i16": i16,
            "sum16": sum_.astype(np.float16),
            "sdm16": sdm.astype(np.float16),
            "sum32": sum_,
            "wm16": wm, "dm32": dm, "dAm32": dAm, "dBm32": dBm,
        })
    res = run_bass_kernel_spmd(nc, in_maps, list(range(N_CORES)),
                               trace=_NC_CACHE.get("trace", False))
    _NC_CACHE["last_result"] = res
    out = np.concatenate([r["out16"] for r in res.results], axis=0)
    return out.astype(np.float32)


if __name__ == "__main__":
    rng = np.random.default_rng(0)
    w = rng.standard_normal((B, C, H, W)).astype(np.float32)
    w[:, 0] = rng.integers(0, 10, (B, H, W)).astype(np.float32)
    out = kernel(w)
    print("ran:", out.shape, out.dtype)
